# revision 1
# baseline (speedup 1.0000x reference)
"""Self-contained Trainium2 Bass kernel for the multi-head attention module.

Sharding: flat 8-way head tensor-parallelism. Core c owns heads {2c, 2c+1}
for both batches; after attention one 8-core AllToAll reshards from
head-space to sequence-space and each core runs the output projection for
its 512 token rows. Host concatenates the per-core row chunks.

All matmuls run in float32r (TF32-like, 1 cyc/row for moving dim >= 256).
"""

import sys

sys.path.insert(0, "/opt/trn_rl_repo")

import numpy as np

from concourse import bacc, bass_utils, mybir, tile
from concourse.masks import make_identity

B, S, D, H, DK, DV, DO = 2, 2048, 1024, 16, 64, 64, 1024
T = B * S          # 4096 flattened tokens
NCORES = 8
HPC = H // NCORES  # 2 heads per core
ROWS = T // NCORES # 512 output rows per core
TCH = 512          # token chunk for projections / q chunks
F32 = mybir.dt.float32
F32R = mybir.dt.float32r
EXP = mybir.ActivationFunctionType.Exp

_cache = {}


def _build(collective=True, phases=4):
    nc = bacc.Bacc("TRN2", target_bir_lowering=False, debug=False,
                   num_devices=NCORES if collective else 1)
    x_d = nc.dram_tensor("x", [T, D], F32R, kind="ExternalInput").ap()
    wq_d = nc.dram_tensor("wq", [D, HPC * DK], F32R, kind="ExternalInput").ap()
    wk_d = nc.dram_tensor("wk", [D, HPC * DK], F32R, kind="ExternalInput").ap()
    wv_d = nc.dram_tensor("wv", [D, HPC * DV], F32R, kind="ExternalInput").ap()
    wo_d = nc.dram_tensor("wo", [H * DV, DO], F32R, kind="ExternalInput").ap()
    out_d = nc.dram_tensor("out", [ROWS, DO], F32, kind="ExternalOutput").ap()
    bnc_in = [nc.dram_tensor(f"bnc_in{h}", [NCORES, 64, ROWS], F32R).ap()
              for h in range(HPC)]
    bnc_out = [nc.dram_tensor(f"bnc_out{h}", [NCORES, 64, ROWS], F32R).ap()
               for h in range(HPC)]

    with tile.TileContext(nc) as tc:
        with (
            tc.tile_pool(name="sb", bufs=1) as sb,
            tc.tile_pool(name="ps", bufs=1, space="PSUM") as ps,
            nc.allow_low_precision(reason="f32r compute is intentional"),
        ):
            # constants
            ident = sb.tile([128, 128], F32, tag="ident", bufs=1)
            make_identity(nc, ident[:])
            ident_r = sb.tile([128, 128], F32R, tag="identr", bufs=1)
            nc.vector.tensor_copy(ident_r[:], ident[:])
            ones_f = sb.tile([128, 64], F32, tag="onesf", bufs=1)
            nc.vector.memset(ones_f[:], 1.0)
            ones_b = sb.tile([128, 64], F32R, tag="ones", bufs=1)
            nc.vector.tensor_copy(ones_b[:], ones_f[:])

            # prefetch first x chunks ahead of weight DMAs (queue order)
            _pre_x = {}
            for tci in range(2):
                xs = []
                for tb in range(4):
                    xstg = sb.tile([128, D], F32R, tag="xstg", bufs=8,
                                   name=f"xstg{tci}_{tb}")
                    row0 = (tci * 4 + tb) * 128
                    nc.sync.dma_start(xstg[:], x_d[row0:row0 + 128, :])
                    xs.append(xstg)
                _pre_x[tci] = xs
            # qkv weights: direct DMA into f32r tiles
            w_r = {}
            for w_d, name in ((wq_d, "q"), (wk_d, "k"), (wv_d, "v")):
                tiles = []
                for dc in range(8):
                    wr = sb.tile([128, 128], F32R, tag=f"w{name}", bufs=8)
                    nc.sync.dma_start(wr[:], w_d[dc * 128:(dc + 1) * 128, :])
                    tiles.append(wr)
                w_r[name] = tiles

            # persistent activations
            qT = sb.tile([128, T], F32R, tag="qT", bufs=1)
            kT = sb.tile([128, T], F32R, tag="kT", bufs=1)
            # v in natural layout per head: 32 t-blocks x [ones | 64 v cols]
            v_aug = []
            for h in range(HPC):
                va = sb.tile([128, 32 * 65], F32R, tag=f"vaug{h}", bufs=1)
                ones_cols = va[:].rearrange("p (b c) -> p b c", c=65)[:, :, 64:65]
                nc.vector.tensor_copy(
                    ones_cols,
                    ones_f[:, 0:32].rearrange("p (a b) -> p a b", b=1))
                v_aug.append(va)

            last_obc = [None]

            # ---- phase 1: stream x, transpose, project q/k/v ----
            def load_xstgs(tci):
                xstgs = []
                for tb in range(4):
                    xstg = sb.tile([128, D], F32R, tag="xstg", bufs=8,
                                   name=f"xstg{tci}_{tb}")
                    row0 = (tci * 4 + tb) * 128
                    nc.sync.dma_start(xstg[:], x_d[row0:row0 + 128, :])
                    xstgs.append(xstg)
                return xstgs

            def emit_tchunk(tci, copies_on_act=True, xstgs=None):
                xTc = [sb.tile([128, TCH], F32R, tag="xTc", bufs=10,
                               name=f"xTc{tci}_{d}") for d in range(8)]
                if xstgs is None:
                    xstgs = load_xstgs(tci)
                for dc in range(8):
                    ptr = ps.tile([128, TCH], F32R, tag="ps_a", bufs=2,
                                  name=f"ptr{tci}_{dc}")
                    for tb in range(4):
                        nc.tensor.transpose(
                            ptr[:, tb * 128:(tb + 1) * 128],
                            xstgs[tb][:, dc * 128:(dc + 1) * 128], ident_r[:])
                    if copies_on_act and dc % 2 == 0:
                        nc.scalar.copy(xTc[dc][:], ptr[:])
                    else:
                        nc.vector.tensor_copy(xTc[dc][:], ptr[:])

                for name in ("q", "k", "v"):
                    pp = ps.tile([128, TCH], F32, tag="ps_a", bufs=2,
                                 name=f"pp{tci}_{name}")
                    for dc in range(8):
                        nc.tensor.matmul(pp[:], w_r[name][dc][:], xTc[dc][:],
                                         start=(dc == 0), stop=(dc == 7))
                    col = tci * TCH
                    if name == "q":
                        nc.vector.tensor_copy(qT[:, col:col + TCH], pp[:])
                    elif name == "k":
                        nc.vector.tensor_copy(kT[:, col:col + TCH], pp[:])
                    else:
                        vTs = sb.tile([128, TCH], F32R, tag="vTs", bufs=2,
                                      name=f"vTs{tci}")
                        nc.vector.tensor_copy(vTs[:], pp[:])
                        for h in range(HPC):
                            for tb in range(4):
                                pv = ps.tile([128, 64], F32R, tag="ps_o",
                                             bufs=2, name=f"pv{tci}_{h}_{tb}")
                                with nc.allow_low_precision(
                                        reason="pure transpose"):
                                    nc.tensor.transpose(
                                        pv[:],
                                        vTs[h * 64:(h + 1) * 64,
                                            tb * 128:(tb + 1) * 128],
                                        ident_r[h * 64:(h + 1) * 64,
                                                h * 64:(h + 1) * 64])
                                blk = tci * 4 + tb
                                nc.vector.tensor_copy(
                                    v_aug[h][:, blk * 65:blk * 65 + 64],
                                    pv[:])

            # ---- phase 2: attention unit for (batch, head, q-chunk) ----
            def emit_attn(b, h, qc):
                qoff = b * S + qc * TCH
                po = ps.tile([65, TCH], F32, tag="ps_o", bufs=2,
                             name=f"po{b}_{h}_{qc}")
                for kb2 in range(S // 256):
                    pscr = ps.tile([128, 2 * TCH], F32, tag="ps_s", bufs=2,
                                   name=f"pscr{b}_{h}_{qc}_{kb2}")
                    for j in range(2):
                        kb = 2 * kb2 + j
                        koff = b * S + kb * 128
                        nc.tensor.matmul(
                            pscr[:, j * TCH:(j + 1) * TCH],
                            kT[h * 64:(h + 1) * 64, koff:koff + 128],
                            qT[h * 64:(h + 1) * 64, qoff:qoff + TCH],
                            start=True, stop=True)
                    ex = sb.tile([128, 2 * TCH], F32R, tag="ex", bufs=4,
                                 name=f"ex{b}_{h}_{qc}_{kb2}")
                    nc.scalar.activation(ex[:], pscr[:], EXP, scale=0.125)
                    for j in range(2):
                        kb = 2 * kb2 + j
                        blk = b * 16 + kb
                        nc.tensor.matmul(
                            po[:],
                            v_aug[h][:, blk * 65:blk * 65 + 65],
                            ex[:, j * TCH:(j + 1) * TCH],
                            start=(kb == 0), stop=(kb == S // 128 - 1))
                # normalize: r = 1/sumexp (row 64), broadcast via PE
                r65 = sb.tile([65, TCH], F32R, tag="r", bufs=2,
                              name=f"r{b}_{h}_{qc}")
                nc.vector.reciprocal(r65[64:65, :], po[64:65, :])
                pbc = ps.tile([64, TCH], F32, tag="ps_o", bufs=2,
                              name=f"pbc{b}_{h}_{qc}")
                nc.tensor.matmul(pbc[:], ones_b[64:65, :],
                                 r65[64:65, :], start=True, stop=True)
                bc_sb = sb.tile([64, TCH], F32R, tag="bcsb", bufs=2,
                                name=f"bcsb{b}_{h}_{qc}")
                nc.vector.tensor_copy(bc_sb[:], pbc[:])
                obc = sb.tile([64, TCH], F32R, tag="obc", bufs=3,
                              name=f"obc{b}_{h}_{qc}")
                nc.vector.tensor_mul(obc[:], po[0:64, :], bc_sb[:])
                shard = b * (S // TCH) + qc
                nc.sync.dma_start(bnc_in[h][shard, :, :], obc[:])
                last_obc[0] = obc

            # batch-0 projections first, then interleave batch-0 attention
            # with batch-1 projections
            for tci in range(4):
                emit_tchunk(tci, xstgs=_pre_x.get(tci))
            units_b0 = [(0, h, qc) for h in range(HPC)
                        for qc in range(S // TCH)]
            if phases >= 2:
                for u in units_b0[0:3]:
                    emit_attn(*u)
            for i, tci in enumerate(range(4, 8)):
                emit_tchunk(tci, copies_on_act=False)
                if phases >= 2:
                    for u in units_b0[3 + i * 2:3 + (i + 1) * 2]:
                        emit_attn(*u)
            def emit_a2a(h):
                if collective:
                    nc.gpsimd.collective_compute(
                        "AllToAll", mybir.AluOpType.bypass,
                        replica_groups=[list(range(NCORES))],
                        ins=[bnc_in[h][:]], outs=[bnc_out[h][:]])
                else:
                    nc.sync.dma_start(bnc_out[h][:], bnc_in[h][:])

            if phases >= 2:
                for h in range(HPC):
                    for qc in range(S // TCH):
                        emit_attn(1, h, qc)
                    if phases >= 3 and h == 0:
                        emit_a2a(0)

            # ---- phase 3: A2A head-space -> sequence-space (2nd half) ----
            if phases >= 3:
                emit_a2a(1)
                # keep PE's HAM clock warm across the exposed collective so
                # the output projection starts at 2.4 GHz
                for wi in range(24):
                    wps = ps.tile([64, TCH], F32, tag="ps_a", bufs=2,
                                  name=f"warm{wi}")
                    nc.tensor.matmul(
                        wps[:], last_obc[0][:, 0:64], last_obc[0][:],
                        start=True, stop=True)

            # ---- phase 4: output projection for our 512 rows ----
            phase4 = phases >= 4
            oTf = []
            for hc in range(8 if phase4 else 0):
                t = sb.tile([128, ROWS], F32R, tag="oTf", bufs=8,
                            name=f"oTf{hc}")
                nc.sync.dma_start(t[0:64, :], bnc_out[0][hc, :, :])
                nc.sync.dma_start(t[64:128, :], bnc_out[1][hc, :, :])
                oTf.append(t)
            wo_r = []
            for hc in range(8 if phase4 else 0):
                wr = sb.tile([128, DO], F32R, tag="xstg", bufs=8,
                             name=f"wo{hc}")
                nc.sync.dma_start(wr[:], wo_d[hc * 128:(hc + 1) * 128, :])
                wo_r.append(wr)
            for sb_i in range(ROWS // 128 if phase4 else 0):
                outt = sb.tile([128, DO], F32, tag="osb", bufs=2)
                for doc in range(DO // 512):
                    pout = ps.tile([128, 512], F32, tag="ps_s", bufs=2)
                    for hc in range(8):
                        nc.tensor.matmul(
                            pout[:],
                            oTf[hc][:, sb_i * 128:(sb_i + 1) * 128],
                            wo_r[hc][:, doc * 512:(doc + 1) * 512],
                            start=(hc == 0), stop=(hc == 7))
                    nc.scalar.copy(outt[:, doc * 512:(doc + 1) * 512], pout[:])
                nc.sync.dma_start(out_d[sb_i * 128:(sb_i + 1) * 128, :],
                                  outt[:])

    nc.compile()
    return nc


def _get_nc():
    if "nc" not in _cache:
        _cache["nc"] = _build()
    return _cache["nc"]


def _in_maps(x, Wq, Wk, Wv, Wo):
    x_flat = np.ascontiguousarray(x.reshape(T, D), dtype=np.float32)
    wo = np.ascontiguousarray(Wo, dtype=np.float32)
    maps = []
    for c in range(NCORES):
        h0, h1 = HPC * c, HPC * c + 1
        maps.append({
            "x": x_flat,
            "wq": np.ascontiguousarray(
                np.concatenate([Wq[h0], Wq[h1]], axis=1), dtype=np.float32),
            "wk": np.ascontiguousarray(
                np.concatenate([Wk[h0], Wk[h1]], axis=1), dtype=np.float32),
            "wv": np.ascontiguousarray(
                np.concatenate([Wv[h0], Wv[h1]], axis=1), dtype=np.float32),
            "wo": wo,
        })
    return maps


def kernel(x, Wq, Wk, Wv, Wo, **_):
    nc = _get_nc()
    res = bass_utils.run_bass_kernel_spmd(
        nc, _in_maps(x, Wq, Wk, Wv, Wo), core_ids=list(range(NCORES)))
    out = np.concatenate([res.results[c]["out"] for c in range(NCORES)],
                         axis=0)
    return out.reshape(B, S, DO)



# revision 4
# speedup vs baseline: 1.1101x; 1.1101x over previous
"""Self-contained Trainium2 Bass kernel for the multi-head attention module.

Sharding: flat 8-way head tensor-parallelism. Core c owns heads {2c, 2c+1}
for both batches; after attention one 8-core AllToAll per head-pair index
reshards from head-space to sequence-space and each core runs the output
projection for its 512 token rows. Host concatenates the per-core row
chunks.

v2 layout: everything bf16 on the matmul paths (1 cyc/row on PE, half the
DMA + collective bytes). x is transposed on the host so the kernel DMAs
[D, T] tiles straight into SBUF: no PE transposes, no staging copies. V is
computed directly in [token, v] layout via xT-stationary matmuls. The
Activation engine runs only the softmax exps (it is the attention-phase
floor); all PSUM->SBUF copies live on DVE. The output projection is split
into an even-heads pass (hidden behind late attention, after the first
AllToAll) and an odd-heads pass (the only work after the second AllToAll).
"""

import sys

sys.path.insert(0, "/opt/trn_rl_repo")

import ml_dtypes
import numpy as np

from concourse import bacc, bass_utils, mybir, tile

B, S, D, H, DK, DV, DO = 2, 2048, 1024, 16, 64, 64, 1024
T = B * S          # 4096 flattened tokens
NCORES = 8
HPC = H // NCORES  # 2 heads per core
ROWS = T // NCORES # 512 output rows per core
TCH = 512          # token chunk for projections / q chunks
F32 = mybir.dt.float32
F32R = mybir.dt.float32r
BF16 = mybir.dt.bfloat16
EXP = mybir.ActivationFunctionType.Exp

_cache = {}


def _build(collective=True):
    nc = bacc.Bacc("TRN2", target_bir_lowering=False, debug=False,
                   num_devices=NCORES if collective else 1)
    xt_d = nc.dram_tensor("xt", [D, T], BF16, kind="ExternalInput").ap()
    wq_d = nc.dram_tensor("wq", [D, HPC * DK], BF16, kind="ExternalInput").ap()
    wk_d = nc.dram_tensor("wk", [D, HPC * DK], BF16, kind="ExternalInput").ap()
    wv_d = nc.dram_tensor("wv", [D, HPC * DV], BF16, kind="ExternalInput").ap()
    wo_d = nc.dram_tensor("wo", [H * DV, DO], BF16, kind="ExternalInput").ap()
    out_d = nc.dram_tensor("out", [ROWS, DO], F32, kind="ExternalOutput").ap()
    bnc_in = [nc.dram_tensor(f"bnc_in{h}", [NCORES, 64, ROWS], BF16).ap()
              for h in range(HPC)]
    bnc_out = [nc.dram_tensor(f"bnc_out{h}", [NCORES, 64, ROWS], BF16).ap()
               for h in range(HPC)]

    with tile.TileContext(nc) as tc:
        with (
            tc.tile_pool(name="sb", bufs=1) as sb,
            tc.tile_pool(name="ps", bufs=1, space="PSUM") as ps,
            nc.allow_low_precision(reason="bf16 compute is intentional"),
        ):
            # constants for the softmax-normalization broadcast matmul
            ones_f = sb.tile([128, 64], F32, tag="onesf", bufs=1)
            nc.vector.memset(ones_f[:], 1.0)
            ones_b = sb.tile([128, 64], F32R, tag="ones", bufs=1)
            nc.vector.tensor_copy(ones_b[:], ones_f[:])

            # qkv weights: direct DMA into bf16 tiles
            w_r = {}
            for w_d, name in ((wq_d, "q"), (wk_d, "k"), (wv_d, "v")):
                tiles = []
                for dc in range(8):
                    wr = sb.tile([128, 128], BF16, tag=f"w{name}", bufs=8)
                    nc.sync.dma_start(wr[:], w_d[dc * 128:(dc + 1) * 128, :])
                    tiles.append(wr)
                w_r[name] = tiles

            # x^T persistent tiles, DMA'd chunk-major so chunk 0 lands first
            xT = [sb.tile([128, T], BF16, tag=f"xT{dc}", bufs=1,
                          name=f"xT{dc}")
                  for dc in range(8)]
            for tci in range(8):
                c0 = tci * TCH
                for dc in range(8):
                    nc.sync.dma_start(
                        xT[dc][:, c0:c0 + TCH],
                        xt_d[dc * 128:(dc + 1) * 128, c0:c0 + TCH])

            # wo pair tiles for the two projection passes: pass h reads heads
            # {4p+h, 4p+2+h} stacked on partitions, matching the oTf layout
            wo_p = {0: [], 1: []}
            for h in range(HPC):
                for p in range(4):
                    wt = sb.tile([128, DO], BF16, tag="wo", bufs=8,
                                 name=f"wo{h}_{p}")
                    for half, head in ((0, 4 * p + h), (1, 4 * p + 2 + h)):
                        nc.sync.dma_start(
                            wt[half * 64:half * 64 + 64, :],
                            wo_d[head * 64:head * 64 + 64, :])
                    wo_p[h].append(wt)

            # persistent activations
            qT = sb.tile([128, T], BF16, tag="qT", bufs=1)
            kT = sb.tile([128, T], BF16, tag="kT", bufs=1)
            # v in natural [token, v] layout: 32 t-blocks x (2 heads x
            # [64 v cols | ones]) -> AV stationary slices [128, 65]
            v_dual = sb.tile([128, 32 * 130], BF16, tag="vdual", bufs=1)
            ones_cols = v_dual[:].rearrange(
                "p (b h c) -> p b h c", h=2, c=65)[:, :, :, 64:65]
            nc.vector.memset(ones_cols, 1.0)

            last_obc = [None]

            # ---- phase 1: project q/k (W stationary) and v (xT stationary)
            def emit_chunk(tci):
                c0 = tci * TCH
                for name in ("q", "k"):
                    pp = ps.tile([128, TCH], F32, tag="ps_a", bufs=2,
                                 name=f"pp{tci}_{name}")
                    for dc in range(8):
                        nc.tensor.matmul(pp[:], w_r[name][dc][:],
                                         xT[dc][:, c0:c0 + TCH],
                                         start=(dc == 0), stop=(dc == 7))
                    dst = qT if name == "q" else kT
                    nc.vector.tensor_copy(dst[:, c0:c0 + TCH], pp[:])
                pv = ps.tile([128, TCH], F32, tag="ps_a", bufs=2,
                             name=f"pv{tci}")
                for tb in range(4):
                    blk = tci * 4 + tb
                    for dc in range(8):
                        nc.tensor.matmul(
                            pv[:, tb * 128:(tb + 1) * 128],
                            xT[dc][:, blk * 128:(blk + 1) * 128],
                            w_r["v"][dc][:],
                            start=(dc == 0), stop=(dc == 7))
                vd = v_dual[:, tci * 4 * 130:(tci + 1) * 4 * 130].rearrange(
                    "p (b h c) -> p b h c", h=2, c=65)[:, :, :, 0:64]
                nc.vector.tensor_copy(
                    vd, pv[:].rearrange("p (b h c) -> p b h c", h=2, c=64))

            # ---- phase 2: attention unit (batch, head, q-chunk) ----
            # The normalization tail of each unit is deferred and emitted
            # after the next unit's first score block so the PE never
            # head-of-line blocks on the DVE reciprocal.
            pending_tail = [None]

            def emit_tail():
                if pending_tail[0] is None:
                    return
                b, h, qc, po = pending_tail[0]
                pending_tail[0] = None
                r65 = sb.tile([65, TCH], F32R, tag="r", bufs=2,
                              name=f"r{b}_{h}_{qc}")
                nc.vector.reciprocal(r65[64:65, :], po[64:65, :])
                pbc = ps.tile([64, TCH], F32, tag="ps_a", bufs=2,
                              name=f"pbc{b}_{h}_{qc}")
                nc.tensor.matmul(pbc[:], ones_b[64:65, :],
                                 r65[64:65, :], start=True, stop=True)
                bc_sb = sb.tile([64, TCH], F32R, tag="bcsb", bufs=2,
                                name=f"bcsb{b}_{h}_{qc}")
                nc.vector.tensor_copy(bc_sb[:], pbc[:])
                obc = sb.tile([64, TCH], BF16, tag="obc", bufs=3,
                              name=f"obc{b}_{h}_{qc}")
                nc.vector.tensor_mul(obc[:], po[0:64, :], bc_sb[:])
                shard = b * (S // TCH) + qc
                nc.sync.dma_start(bnc_in[h][shard, :, :], obc[:])
                last_obc[0] = obc

            def emit_attn(b, h, qc):
                qoff = b * S + qc * TCH
                po = ps.tile([65, TCH], F32, tag="ps_o", bufs=2,
                             name=f"po{b}_{h}_{qc}")
                for kb2 in range(S // 256):
                    pscr = ps.tile([128, 2 * TCH], F32, tag="ps_s", bufs=2,
                                   name=f"pscr{b}_{h}_{qc}_{kb2}")
                    for j in range(2):
                        kb = 2 * kb2 + j
                        koff = b * S + kb * 128
                        nc.tensor.matmul(
                            pscr[:, j * TCH:(j + 1) * TCH],
                            kT[h * 64:(h + 1) * 64, koff:koff + 128],
                            qT[h * 64:(h + 1) * 64, qoff:qoff + TCH],
                            start=True, stop=True)
                    if kb2 == 0:
                        emit_tail()
                    ex = sb.tile([128, 2 * TCH], BF16, tag="ex", bufs=4,
                                 name=f"ex{b}_{h}_{qc}_{kb2}")
                    nc.scalar.activation(ex[:], pscr[:], EXP, scale=0.125)
                    for j in range(2):
                        kb = 2 * kb2 + j
                        blk = b * 16 + kb
                        nc.tensor.matmul(
                            po[:],
                            v_dual[:, blk * 130 + h * 65:
                                   blk * 130 + h * 65 + 65],
                            ex[:, j * TCH:(j + 1) * TCH],
                            start=(kb == 0), stop=(kb == S // 128 - 1))
                pending_tail[0] = (b, h, qc, po)

            def emit_a2a(h):
                if collective:
                    nc.gpsimd.collective_compute(
                        "AllToAll", mybir.AluOpType.bypass,
                        replica_groups=[list(range(NCORES))],
                        ins=[bnc_in[h][:]], outs=[bnc_out[h][:]])
                else:
                    nc.sync.dma_start(bnc_out[h][:], bnc_in[h][:])

            # ---- output projection pass h: heads {4p+h, 4p+2+h} ----
            oTf = {}

            def emit_oTf(h):
                tiles = []
                for p in range(4):
                    t = sb.tile([128, ROWS], BF16, tag="oTf", bufs=8,
                                name=f"oTf{h}_{p}")
                    nc.sync.dma_start(t[0:64, :], bnc_out[h][2 * p, :, :])
                    nc.sync.dma_start(t[64:128, :], bnc_out[h][2 * p + 1, :, :])
                    tiles.append(t)
                oTf[h] = tiles

            o0sb = [sb.tile([128, 512], BF16, tag="o0sb", bufs=8,
                            name=f"o0sb{i}") for i in range(8)]

            def emit_pass0_chunk(ci):
                sbi, doc = divmod(ci, 2)
                pout = ps.tile([128, 512], F32, tag="ps_a", bufs=2,
                               name=f"p0_{ci}")
                for p in range(4):
                    nc.tensor.matmul(
                        pout[:],
                        oTf[0][p][:, sbi * 128:(sbi + 1) * 128],
                        wo_p[0][p][:, doc * 512:(doc + 1) * 512],
                        start=(p == 0), stop=(p == 3))
                nc.vector.tensor_copy(o0sb[ci][:], pout[:])

            def emit_pass1():
                for sbi in range(4):
                    outt = sb.tile([128, DO], F32, tag="osb", bufs=2,
                                   name=f"outt{sbi}")
                    for doc in range(2):
                        pout = ps.tile([128, 512], F32, tag="ps_a", bufs=2,
                                       name=f"p1_{sbi}_{doc}")
                        for p in range(4):
                            nc.tensor.matmul(
                                pout[:],
                                oTf[1][p][:, sbi * 128:(sbi + 1) * 128],
                                wo_p[1][p][:, doc * 512:(doc + 1) * 512],
                                start=(p == 0), stop=(p == 3))
                        nc.vector.tensor_add(
                            outt[:, doc * 512:(doc + 1) * 512], pout[:],
                            o0sb[sbi * 2 + doc][:])
                    nc.sync.dma_start(out_d[sbi * 128:(sbi + 1) * 128, :],
                                      outt[:])

            # ---- schedule ----
            # chunks 0-3, then batch-0 attention interleaved with chunks 4-7
            for tci in range(4):
                emit_chunk(tci)
            units_b0 = [(0, h, qc) for h in range(HPC)
                        for qc in range(S // TCH)]
            for i, tci in enumerate(range(4, 8)):
                for u in units_b0[2 * i:2 * i + 2]:
                    emit_attn(*u)
                emit_chunk(tci)
            # batch-1 h0 units; A2A(h0); h1 units with pass0 interleaved
            for qc in range(S // TCH):
                emit_attn(1, 0, qc)
            emit_tail()  # flush (1,0,3)'s obc before the h0 collective
            emit_a2a(0)
            emit_oTf(0)
            for qc in range(S // TCH):
                emit_attn(1, 1, qc)
                emit_pass0_chunk(2 * qc)
                emit_pass0_chunk(2 * qc + 1)
            emit_tail()
            emit_a2a(1)
            # keep PE's HAM clock warm across the exposed collective so the
            # odd-heads projection pass starts at 2.4 GHz
            for wi in range(12):
                wps = ps.tile([64, TCH], F32, tag="ps_a", bufs=2,
                              name=f"warm{wi}")
                nc.tensor.matmul(
                    wps[:], last_obc[0][:, 0:64], last_obc[0][:],
                    start=True, stop=True)
            emit_oTf(1)
            emit_pass1()

    nc.compile()
    return nc


def _get_nc():
    if "nc" not in _cache:
        _cache["nc"] = _build()
    return _cache["nc"]


def _in_maps(x, Wq, Wk, Wv, Wo):
    bf16 = ml_dtypes.bfloat16
    xt = np.ascontiguousarray(
        x.reshape(T, D).T.astype(bf16))
    wo = np.ascontiguousarray(Wo.astype(bf16))
    maps = []
    for c in range(NCORES):
        h0, h1 = HPC * c, HPC * c + 1
        maps.append({
            "xt": xt,
            "wq": np.ascontiguousarray(
                np.concatenate([Wq[h0], Wq[h1]], axis=1).astype(bf16)),
            "wk": np.ascontiguousarray(
                np.concatenate([Wk[h0], Wk[h1]], axis=1).astype(bf16)),
            "wv": np.ascontiguousarray(
                np.concatenate([Wv[h0], Wv[h1]], axis=1).astype(bf16)),
            "wo": wo,
        })
    return maps


def kernel(x, Wq, Wk, Wv, Wo, **_):
    nc = _get_nc()
    res = bass_utils.run_bass_kernel_spmd(
        nc, _in_maps(x, Wq, Wk, Wv, Wo), core_ids=list(range(NCORES)))
    out = np.concatenate([res.results[c]["out"] for c in range(NCORES)],
                         axis=0)
    return out.reshape(B, S, DO)


# revision 8
# speedup vs baseline: 1.1242x; 1.0128x over previous
"""Self-contained Trainium2 Bass kernel for the multi-head attention module.

Sharding: flat 8-way head tensor-parallelism. Core c owns heads {2c, 2c+1}
for both batches; after attention one 8-core AllToAll per head-pair index
reshards from head-space to sequence-space and each core runs the output
projection for its 512 token rows. Host concatenates the per-core row
chunks.

v2 layout: everything bf16 on the matmul paths (1 cyc/row on PE, half the
DMA + collective bytes). x is transposed on the host so the kernel DMAs
[D, T] tiles straight into SBUF: no PE transposes, no staging copies. V is
computed directly in [token, v] layout via xT-stationary matmuls. The
Activation engine runs only the softmax exps (it is the attention-phase
floor); all PSUM->SBUF copies live on DVE. The output projection is split
into an even-heads pass (hidden behind late attention, after the first
AllToAll) and an odd-heads pass (the only work after the second AllToAll).
"""

import sys

sys.path.insert(0, "/opt/trn_rl_repo")

import ml_dtypes
import numpy as np

from concourse import bacc, bass_utils, mybir, tile

B, S, D, H, DK, DV, DO = 2, 2048, 1024, 16, 64, 64, 1024
T = B * S          # 4096 flattened tokens
NCORES = 8
HPC = H // NCORES  # 2 heads per core
ROWS = T // NCORES # 512 output rows per core
TCH = 512          # token chunk for projections / q chunks
F32 = mybir.dt.float32
F32R = mybir.dt.float32r
BF16 = mybir.dt.bfloat16
EXP = mybir.ActivationFunctionType.Exp

_cache = {}


def _build(collective=True):
    nc = bacc.Bacc("TRN2", target_bir_lowering=False, debug=False,
                   num_devices=NCORES if collective else 1)
    xt_d = nc.dram_tensor("xt", [D, T], BF16, kind="ExternalInput").ap()
    wq_d = nc.dram_tensor("wq", [D, HPC * DK], BF16, kind="ExternalInput").ap()
    wk_d = nc.dram_tensor("wk", [D, HPC * DK], BF16, kind="ExternalInput").ap()
    wv_d = nc.dram_tensor("wv", [D, HPC * DV], BF16, kind="ExternalInput").ap()
    wo_d = nc.dram_tensor("wo", [H * DV, DO], BF16, kind="ExternalInput").ap()
    out_d = nc.dram_tensor("out", [ROWS, DO], F32, kind="ExternalOutput").ap()
    bnc_in = [nc.dram_tensor(f"bnc_in{h}", [NCORES, 64, ROWS], BF16).ap()
              for h in range(HPC)]
    bnc_out = [nc.dram_tensor(f"bnc_out{h}", [NCORES, 64, ROWS], BF16).ap()
               for h in range(HPC)]

    with tile.TileContext(nc) as tc:
        with (
            tc.tile_pool(name="sb", bufs=1) as sb,
            tc.tile_pool(name="ps", bufs=1, space="PSUM") as ps,
            nc.allow_low_precision(reason="bf16 compute is intentional"),
        ):
            # constants for the softmax-normalization broadcast matmul
            ones_f = sb.tile([128, 64], F32, tag="onesf", bufs=1)
            nc.vector.memset(ones_f[:], 1.0)
            ones_b = sb.tile([128, 64], F32R, tag="ones", bufs=1)
            nc.vector.tensor_copy(ones_b[:], ones_f[:])

            # qkv weights: direct DMA into bf16 tiles
            w_r = {}
            for w_d, name in ((wq_d, "q"), (wk_d, "k"), (wv_d, "v")):
                tiles = []
                for dc in range(8):
                    wr = sb.tile([128, 128], BF16, tag=f"w{name}", bufs=8)
                    nc.sync.dma_start(wr[:], w_d[dc * 128:(dc + 1) * 128, :])
                    tiles.append(wr)
                w_r[name] = tiles

            # x^T persistent tiles, one per (d-block, chunk) so a chunk's
            # matmuls only wait on that chunk's DMAs (DMA-written tiles get
            # tile-granular deps), DMA'd chunk-major so chunk 0 lands first
            xT = [[None] * 8 for _ in range(8)]
            for tci in range(8):
                c0 = tci * TCH
                for dc in range(8):
                    t = sb.tile([128, TCH], BF16, tag=f"xT{dc}", bufs=8,
                                name=f"xT{dc}_{tci}")
                    nc.sync.dma_start(t[:], xt_d[dc * 128:(dc + 1) * 128,
                                                 c0:c0 + TCH])
                    xT[dc][tci] = t

            # wo pair tiles for the two projection passes: pass h reads heads
            # {4p+h, 4p+2+h} stacked on partitions, matching the oTf layout
            wo_p = {0: [], 1: []}
            for h in range(HPC):
                for p in range(4):
                    wt = sb.tile([128, DO], BF16, tag="wo", bufs=8,
                                 name=f"wo{h}_{p}")
                    for half, head in ((0, 4 * p + h), (1, 4 * p + 2 + h)):
                        nc.sync.dma_start(
                            wt[half * 64:half * 64 + 64, :],
                            wo_d[head * 64:head * 64 + 64, :])
                    wo_p[h].append(wt)

            # persistent activations
            qT = sb.tile([128, T], BF16, tag="qT", bufs=1)
            kT = sb.tile([128, T], BF16, tag="kT", bufs=1)
            # v in natural [token, v] layout: 32 t-blocks x (2 heads x
            # [64 v cols | ones]) -> AV stationary slices [128, 65]
            v_dual = sb.tile([128, 32 * 130], BF16, tag="vdual", bufs=1)
            ones_cols = v_dual[:].rearrange(
                "p (b h c) -> p b h c", h=2, c=65)[:, :, :, 64:65]
            nc.vector.memset(ones_cols, 1.0)

            last_obc = [None]

            # ---- phase 1: project q/k (W stationary) and v (xT stationary)
            def emit_proj(tci, which):
                c0 = tci * TCH
                for name in ("q", "k"):
                    if name not in which:
                        continue
                    pp = ps.tile([128, TCH], F32, tag="ps_a", bufs=2,
                                 name=f"pp{tci}_{name}")
                    for dc in range(8):
                        nc.tensor.matmul(pp[:], w_r[name][dc][:],
                                         xT[dc][tci][:],
                                         start=(dc == 0), stop=(dc == 7))
                    dst = qT if name == "q" else kT
                    nc.vector.tensor_copy(dst[:, c0:c0 + TCH], pp[:])
                if "v" not in which:
                    return
                pv = ps.tile([128, TCH], F32, tag="ps_a", bufs=2,
                             name=f"pv{tci}")
                for tb in range(4):
                    for dc in range(8):
                        nc.tensor.matmul(
                            pv[:, tb * 128:(tb + 1) * 128],
                            xT[dc][tci][:, tb * 128:(tb + 1) * 128],
                            w_r["v"][dc][:],
                            start=(dc == 0), stop=(dc == 7))
                vd = v_dual[:, tci * 4 * 130:(tci + 1) * 4 * 130].rearrange(
                    "p (b h c) -> p b h c", h=2, c=65)[:, :, :, 0:64]
                nc.vector.tensor_copy(
                    vd, pv[:].rearrange("p (b h c) -> p b h c", h=2, c=64))

            # ---- phase 2: attention unit (batch, head, q-chunk) ----
            # The normalization tail of each unit is deferred and emitted
            # after the next unit's first score block so the PE never
            # head-of-line blocks on the DVE reciprocal.
            pending_tail = [None]

            def emit_tail():
                if pending_tail[0] is None:
                    return
                b, h, qc, po = pending_tail[0]
                pending_tail[0] = None
                r65 = sb.tile([65, TCH], F32R, tag="r", bufs=2,
                              name=f"r{b}_{h}_{qc}")
                nc.vector.reciprocal(r65[64:65, :], po[64:65, :])
                pbc = ps.tile([64, TCH], F32, tag="ps_a", bufs=2,
                              name=f"pbc{b}_{h}_{qc}")
                nc.tensor.matmul(pbc[:], ones_b[64:65, :],
                                 r65[64:65, :], start=True, stop=True)
                bc_sb = sb.tile([64, TCH], F32R, tag="bcsb", bufs=2,
                                name=f"bcsb{b}_{h}_{qc}")
                nc.vector.tensor_copy(bc_sb[:], pbc[:])
                obc = sb.tile([64, TCH], BF16, tag="obc", bufs=3,
                              name=f"obc{b}_{h}_{qc}")
                nc.vector.tensor_mul(obc[:], po[0:64, :], bc_sb[:])
                shard = b * (S // TCH) + qc
                nc.sync.dma_start(bnc_in[h][shard, :, :], obc[:])
                last_obc[0] = obc

            def emit_attn(b, h, qc):
                qoff = b * S + qc * TCH
                po = ps.tile([65, TCH], F32, tag="ps_o", bufs=2,
                             name=f"po{b}_{h}_{qc}")
                for kb2 in range(S // 256):
                    pscr = ps.tile([128, 2 * TCH], F32, tag="ps_s", bufs=2,
                                   name=f"pscr{b}_{h}_{qc}_{kb2}")
                    for j in range(2):
                        kb = 2 * kb2 + j
                        koff = b * S + kb * 128
                        nc.tensor.matmul(
                            pscr[:, j * TCH:(j + 1) * TCH],
                            kT[h * 64:(h + 1) * 64, koff:koff + 128],
                            qT[h * 64:(h + 1) * 64, qoff:qoff + TCH],
                            start=True, stop=True)
                    if kb2 == 0:
                        emit_tail()
                    ex = sb.tile([128, 2 * TCH], BF16, tag="ex", bufs=4,
                                 name=f"ex{b}_{h}_{qc}_{kb2}")
                    nc.scalar.activation(ex[:], pscr[:], EXP, scale=0.125)
                    for j in range(2):
                        kb = 2 * kb2 + j
                        blk = b * 16 + kb
                        nc.tensor.matmul(
                            po[:],
                            v_dual[:, blk * 130 + h * 65:
                                   blk * 130 + h * 65 + 65],
                            ex[:, j * TCH:(j + 1) * TCH],
                            start=(kb == 0), stop=(kb == S // 128 - 1))
                pending_tail[0] = (b, h, qc, po)

            def emit_a2a(h):
                if collective:
                    nc.gpsimd.collective_compute(
                        "AllToAll", mybir.AluOpType.bypass,
                        replica_groups=[list(range(NCORES))],
                        ins=[bnc_in[h][:]], outs=[bnc_out[h][:]])
                else:
                    nc.sync.dma_start(bnc_out[h][:], bnc_in[h][:])

            # ---- output projection pass h: heads {4p+h, 4p+2+h} ----
            oTf = {}

            def emit_oTf(h):
                tiles = []
                for p in range(4):
                    t = sb.tile([128, ROWS], BF16, tag="oTf", bufs=8,
                                name=f"oTf{h}_{p}")
                    nc.sync.dma_start(t[0:64, :], bnc_out[h][2 * p, :, :])
                    nc.sync.dma_start(t[64:128, :], bnc_out[h][2 * p + 1, :, :])
                    tiles.append(t)
                oTf[h] = tiles

            o0sb = [sb.tile([128, 512], BF16, tag="o0sb", bufs=8,
                            name=f"o0sb{i}") for i in range(8)]

            def emit_pass0_chunk(ci):
                sbi, doc = divmod(ci, 2)
                pout = ps.tile([128, 512], F32, tag="ps_a", bufs=2,
                               name=f"p0_{ci}")
                for p in range(4):
                    nc.tensor.matmul(
                        pout[:],
                        oTf[0][p][:, sbi * 128:(sbi + 1) * 128],
                        wo_p[0][p][:, doc * 512:(doc + 1) * 512],
                        start=(p == 0), stop=(p == 3))
                nc.vector.tensor_copy(o0sb[ci][:], pout[:])

            def emit_pass1():
                for sbi in range(4):
                    outt = sb.tile([128, DO], F32, tag="osb", bufs=2,
                                   name=f"outt{sbi}")
                    for doc in range(2):
                        pout = ps.tile([128, 512], F32, tag="ps_a", bufs=2,
                                       name=f"p1_{sbi}_{doc}")
                        for p in range(4):
                            nc.tensor.matmul(
                                pout[:],
                                oTf[1][p][:, sbi * 128:(sbi + 1) * 128],
                                wo_p[1][p][:, doc * 512:(doc + 1) * 512],
                                start=(p == 0), stop=(p == 3))
                        nc.vector.tensor_add(
                            outt[:, doc * 512:(doc + 1) * 512], pout[:],
                            o0sb[sbi * 2 + doc][:])
                    nc.sync.dma_start(out_d[sbi * 128:(sbi + 1) * 128, :],
                                      outt[:])

            # ---- schedule ----
            # Warm the PE clock while the first DMAs land.
            wrm_f = sb.tile([1, TCH], F32, tag="wrmf", bufs=1)
            nc.vector.memset(wrm_f[:], 1.0)
            wrm = sb.tile([1, TCH], F32R, tag="wrm", bufs=1)
            nc.vector.tensor_copy(wrm[:], wrm_f[:])
            for wi in range(8):
                wps = ps.tile([64, TCH], F32, tag="ps_a", bufs=2,
                              name=f"swarm{wi}")
                nc.tensor.matmul(wps[:], ones_b[0:1, :], wrm[:],
                                 start=True, stop=True)
            # chunks 0-3 fully; batch-0 attention interleaved with the k/v
            # parts of chunks 4-7 (batch-1 q parts are deferred into the
            # otherwise Act-bound batch-1 head-0 region).
            for tci in range(4):
                emit_proj(tci, "qkv")
            units_b0 = [(0, h, qc) for h in range(HPC)
                        for qc in range(S // TCH)]
            for i, tci in enumerate(range(4, 8)):
                for u in units_b0[2 * i:2 * i + 2]:
                    emit_attn(*u)
                emit_proj(tci, "kv")
            emit_proj(4, "q")
            # batch-1 h0 units; A2A(h0); h1 units with pass0 interleaved
            for qc in range(S // TCH):
                emit_attn(1, 0, qc)
                if qc < 3:
                    emit_proj(5 + qc, "q")
            emit_tail()  # flush (1,0,3)'s obc before the h0 collective
            emit_a2a(0)
            emit_oTf(0)
            for qc in range(S // TCH):
                emit_attn(1, 1, qc)
                emit_pass0_chunk(2 * qc)
                emit_pass0_chunk(2 * qc + 1)
            emit_tail()
            emit_a2a(1)
            # keep PE's HAM clock warm across the exposed collective so the
            # odd-heads projection pass starts at 2.4 GHz
            for wi in range(12):
                wps = ps.tile([64, TCH], F32, tag="ps_a", bufs=2,
                              name=f"warm{wi}")
                nc.tensor.matmul(
                    wps[:], last_obc[0][:, 0:64], last_obc[0][:],
                    start=True, stop=True)
            emit_oTf(1)
            emit_pass1()

    nc.compile()
    return nc


def _get_nc():
    if "nc" not in _cache:
        _cache["nc"] = _build()
    return _cache["nc"]


def _in_maps(x, Wq, Wk, Wv, Wo):
    bf16 = ml_dtypes.bfloat16
    xt = np.ascontiguousarray(
        x.reshape(T, D).T.astype(bf16))
    wo = np.ascontiguousarray(Wo.astype(bf16))
    maps = []
    for c in range(NCORES):
        h0, h1 = HPC * c, HPC * c + 1
        maps.append({
            "xt": xt,
            "wq": np.ascontiguousarray(
                np.concatenate([Wq[h0], Wq[h1]], axis=1).astype(bf16)),
            "wk": np.ascontiguousarray(
                np.concatenate([Wk[h0], Wk[h1]], axis=1).astype(bf16)),
            "wv": np.ascontiguousarray(
                np.concatenate([Wv[h0], Wv[h1]], axis=1).astype(bf16)),
            "wo": wo,
        })
    return maps


def kernel(x, Wq, Wk, Wv, Wo, **_):
    nc = _get_nc()
    res = bass_utils.run_bass_kernel_spmd(
        nc, _in_maps(x, Wq, Wk, Wv, Wo), core_ids=list(range(NCORES)))
    out = np.concatenate([res.results[c]["out"] for c in range(NCORES)],
                         axis=0)
    return out.reshape(B, S, DO)


# revision 14
# speedup vs baseline: 1.1979x; 1.0655x over previous
"""Self-contained Trainium2 Bass kernel for the multi-head attention module.

Sharding: flat 8-way head tensor-parallelism. Core c owns heads {2c, 2c+1}
for both batches; after attention one 8-core AllToAll per head-pair index
reshards from head-space to sequence-space and each core runs the output
projection for its 512 token rows. Host concatenates the per-core row
chunks.

v2 layout: everything bf16 on the matmul paths (1 cyc/row on PE, half the
DMA + collective bytes). x is transposed on the host so the kernel DMAs
[D, T] tiles straight into SBUF: no PE transposes, no staging copies. V is
computed directly in [token, v] layout via xT-stationary matmuls. The
Activation engine runs only the softmax exps (it is the attention-phase
floor); all PSUM->SBUF copies live on DVE. The output projection is split
into an even-heads pass (hidden behind late attention, after the first
AllToAll) and an odd-heads pass (the only work after the second AllToAll).
"""

import sys

sys.path.insert(0, "/opt/trn_rl_repo")

import ml_dtypes
import numpy as np

from concourse import bacc, bass_utils, mybir, tile

B, S, D, H, DK, DV, DO = 2, 2048, 1024, 16, 64, 64, 1024
T = B * S          # 4096 flattened tokens
NCORES = 8
HPC = H // NCORES  # 2 heads per core
ROWS = T // NCORES # 512 output rows per core
TCH = 512          # token chunk for projections / q chunks
F32 = mybir.dt.float32
F32R = mybir.dt.float32r
BF16 = mybir.dt.bfloat16
EXP = mybir.ActivationFunctionType.Exp

_cache = {}


def _build(collective=True):
    nc = bacc.Bacc("TRN2", target_bir_lowering=False, debug=False,
                   num_devices=NCORES if collective else 1)
    xt_d = nc.dram_tensor("xt", [D, T], BF16, kind="ExternalInput").ap()
    wq_d = nc.dram_tensor("wq", [D, HPC * DK], BF16, kind="ExternalInput").ap()
    wk_d = nc.dram_tensor("wk", [D, HPC * DK], BF16, kind="ExternalInput").ap()
    wv_d = nc.dram_tensor("wv", [D, HPC * DV], BF16, kind="ExternalInput").ap()
    wo_d = nc.dram_tensor("wo", [H * DV, DO], BF16, kind="ExternalInput").ap()
    out_d = nc.dram_tensor("out", [ROWS, DO], F32, kind="ExternalOutput").ap()
    bnc_in = [nc.dram_tensor(f"bnc_in{h}", [NCORES, 64, ROWS], BF16).ap()
              for h in range(HPC)]
    bnc_out = [nc.dram_tensor(f"bnc_out{h}", [NCORES, 64, ROWS], BF16).ap()
               for h in range(HPC)]

    with tile.TileContext(nc) as tc:
        with (
            tc.tile_pool(name="sb", bufs=1) as sb,
            tc.tile_pool(name="ps", bufs=1, space="PSUM") as ps,
            nc.allow_low_precision(reason="bf16 compute is intentional"),
        ):
            # constants for the softmax-normalization broadcast matmul
            ones_f = sb.tile([128, 64], F32, tag="onesf", bufs=1)
            nc.vector.memset(ones_f[:], 1.0)
            ones_b = sb.tile([128, 64], F32R, tag="ones", bufs=1)
            nc.vector.tensor_copy(ones_b[:], ones_f[:])

            # HWDGE descriptor generation costs ~625ns per DMA instruction,
            # serialized, so inputs are fetched with as few fat strided DMAs
            # as possible. The first x chunk is interleaved with the weights
            # so phase 1 can start ~4us in.
            # qkv weights: one DMA each into a [128, 8*128] tile whose
            # column blocks are the 8 d-blocks (w_sb[p, dc*128+e] =
            # w_d[dc*128+p, e]).
            w_sb = {}

            def load_w(name):
                w_d = {"q": wq_d, "k": wk_d, "v": wv_d}[name]
                wt = sb.tile([128, 8 * 128], BF16, tag=f"w{name}", bufs=1,
                             name=f"w{name}")
                nc.sync.dma_start(
                    wt[:].rearrange("p (dc e) -> p dc e", e=128),
                    w_d[:].rearrange("(dc p) e -> p dc e", p=128))
                w_sb[name] = wt

            # x^T: one [128, 8*512] tile per chunk (column blocks are the 8
            # d-blocks), loaded in two half DMAs (xc[p, dc*512+t] =
            # xt_d[dc*128+p, c0+t]).
            xTc = []
            for tci in range(8):
                t = sb.tile([128, 8 * TCH], BF16, tag="xTc", bufs=8,
                            name=f"xTc{tci}")
                xTc.append(t)

            def load_x_chunk(tci, half):
                c0 = tci * TCH
                dc0 = half * 4
                nc.sync.dma_start(
                    xTc[tci][:, dc0 * TCH:(dc0 + 4) * TCH].rearrange(
                        "p (dc t) -> p dc t", t=TCH),
                    xt_d[dc0 * 128:(dc0 + 4) * 128, c0:c0 + TCH].rearrange(
                        "(dc p) t -> p dc t", p=128))

            load_x_chunk(0, 0)
            load_w("q")
            load_x_chunk(0, 1)
            load_w("k")
            load_w("v")
            for tci in range(1, 8):
                load_x_chunk(tci, 0)
                load_x_chunk(tci, 1)

            def xT(dc, tci):
                return xTc[tci][:, dc * TCH:(dc + 1) * TCH]

            # wo pair tiles for the two projection passes: pass h reads heads
            # {4p+h, 4p+2+h} stacked on partitions, matching the oTf layout
            wo_p = {0: [], 1: []}
            for h in range(HPC):
                for p in range(4):
                    wt = sb.tile([128, DO], BF16, tag="wo", bufs=8,
                                 name=f"wo{h}_{p}")
                    for half, head in ((0, 4 * p + h), (1, 4 * p + 2 + h)):
                        nc.sync.dma_start(
                            wt[half * 64:half * 64 + 64, :],
                            wo_d[head * 64:head * 64 + 64, :])
                    wo_p[h].append(wt)

            # persistent activations
            qT = sb.tile([128, T], BF16, tag="qT", bufs=1)
            kT = sb.tile([128, T], BF16, tag="kT", bufs=1)
            # v in natural [token, v] layout: 32 t-blocks x (2 heads x
            # [64 v cols | ones]) -> AV stationary slices [128, 65]
            v_dual = sb.tile([128, 32 * 130], BF16, tag="vdual", bufs=1)
            ones_cols = v_dual[:].rearrange(
                "p (b h c) -> p b h c", h=2, c=65)[:, :, :, 64:65]
            nc.vector.memset(ones_cols, 1.0)

            last_obc = [None]

            # ---- phase 1: project q/k (W stationary) and v (xT stationary)
            def emit_proj(tci, which):
                c0 = tci * TCH
                for name in ("q", "k"):
                    if name not in which:
                        continue
                    pp = ps.tile([128, TCH], F32, tag="ps_a", bufs=2,
                                 name=f"pp{tci}_{name}")
                    for dc in range(8):
                        nc.tensor.matmul(pp[:],
                                         w_sb[name][:, dc * 128:(dc + 1) * 128],
                                         xT(dc, tci),
                                         start=(dc == 0), stop=(dc == 7))
                    dst = qT if name == "q" else kT
                    nc.vector.tensor_copy(dst[:, c0:c0 + TCH], pp[:])
                if "v" not in which:
                    return
                pv = ps.tile([128, TCH], F32, tag="ps_a", bufs=2,
                             name=f"pv{tci}")
                for tb in range(4):
                    for dc in range(8):
                        nc.tensor.matmul(
                            pv[:, tb * 128:(tb + 1) * 128],
                            xTc[tci][:, dc * TCH + tb * 128:
                                      dc * TCH + (tb + 1) * 128],
                            w_sb["v"][:, dc * 128:(dc + 1) * 128],
                            start=(dc == 0), stop=(dc == 7))
                vd = v_dual[:, tci * 4 * 130:(tci + 1) * 4 * 130].rearrange(
                    "p (b h c) -> p b h c", h=2, c=65)[:, :, :, 0:64]
                nc.vector.tensor_copy(
                    vd, pv[:].rearrange("p (b h c) -> p b h c", h=2, c=64))

            # ---- phase 2: attention unit (batch, head, q-chunk) ----
            # The normalization tail of each unit is deferred and emitted
            # after the next unit's first score block so the PE never
            # head-of-line blocks on the DVE reciprocal.
            pending_tail = [None]

            def emit_tail():
                if pending_tail[0] is None:
                    return
                b, h, qc, po = pending_tail[0]
                pending_tail[0] = None
                r65 = sb.tile([65, TCH], F32R, tag="r", bufs=2,
                              name=f"r{b}_{h}_{qc}")
                nc.vector.reciprocal(r65[64:65, :], po[64:65, :])
                pbc = ps.tile([64, TCH], F32, tag="ps_a", bufs=2,
                              name=f"pbc{b}_{h}_{qc}")
                nc.tensor.matmul(pbc[:], ones_b[64:65, :],
                                 r65[64:65, :], start=True, stop=True)
                bc_sb = sb.tile([64, TCH], F32R, tag="bcsb", bufs=2,
                                name=f"bcsb{b}_{h}_{qc}")
                nc.vector.tensor_copy(bc_sb[:], pbc[:])
                obc = sb.tile([64, TCH], BF16, tag="obc", bufs=3,
                              name=f"obc{b}_{h}_{qc}")
                nc.vector.tensor_mul(obc[:], po[0:64, :], bc_sb[:])
                shard = b * (S // TCH) + qc
                nc.sync.dma_start(bnc_in[h][shard, :, :], obc[:])
                last_obc[0] = obc

            def emit_attn(b, h, qc):
                qoff = b * S + qc * TCH
                po = ps.tile([65, TCH], F32, tag="ps_o", bufs=2,
                             name=f"po{b}_{h}_{qc}")
                for kb2 in range(S // 256):
                    pscr = ps.tile([128, 2 * TCH], F32, tag="ps_s", bufs=2,
                                   name=f"pscr{b}_{h}_{qc}_{kb2}")
                    for j in range(2):
                        kb = 2 * kb2 + j
                        koff = b * S + kb * 128
                        nc.tensor.matmul(
                            pscr[:, j * TCH:(j + 1) * TCH],
                            kT[h * 64:(h + 1) * 64, koff:koff + 128],
                            qT[h * 64:(h + 1) * 64, qoff:qoff + TCH],
                            start=True, stop=True)
                    if kb2 == 0:
                        emit_tail()
                    ex = sb.tile([128, 2 * TCH], BF16, tag="ex", bufs=4,
                                 name=f"ex{b}_{h}_{qc}_{kb2}")
                    nc.scalar.activation(ex[:], pscr[:], EXP, scale=0.125)
                    for j in range(2):
                        kb = 2 * kb2 + j
                        blk = b * 16 + kb
                        nc.tensor.matmul(
                            po[:],
                            v_dual[:, blk * 130 + h * 65:
                                   blk * 130 + h * 65 + 65],
                            ex[:, j * TCH:(j + 1) * TCH],
                            start=(kb == 0), stop=(kb == S // 128 - 1))
                pending_tail[0] = (b, h, qc, po)

            def emit_a2a(h):
                if collective:
                    nc.gpsimd.collective_compute(
                        "AllToAll", mybir.AluOpType.bypass,
                        replica_groups=[list(range(NCORES))],
                        ins=[bnc_in[h][:]], outs=[bnc_out[h][:]])
                else:
                    nc.sync.dma_start(bnc_out[h][:], bnc_in[h][:])

            # ---- output projection pass h: heads {4p+h, 4p+2+h} ----
            # oTf[h][j*64+r, p*512+t] = bnc_out[h][2p+j, r, t]: one tile per
            # head, two fat DMAs (one per 64-partition half)
            oTf = {}

            def emit_oTf(h):
                t = sb.tile([128, 4 * ROWS], BF16, tag="oTf", bufs=2,
                            name=f"oTf{h}")
                for j in range(2):
                    nc.sync.dma_start(
                        t[64 * j:64 * j + 64, :].rearrange(
                            "r (p tt) -> r p tt", tt=ROWS),
                        bnc_out[h][:].rearrange(
                            "(p j) r tt -> j r p tt", j=2)[j])
                oTf[h] = t

            o0sb = [sb.tile([128, 512], BF16, tag="o0sb", bufs=8,
                            name=f"o0sb{i}") for i in range(8)]

            def emit_pass0_chunk(ci):
                sbi, doc = divmod(ci, 2)
                pout = ps.tile([128, 512], F32, tag="ps_a", bufs=2,
                               name=f"p0_{ci}")
                for p in range(4):
                    nc.tensor.matmul(
                        pout[:],
                        oTf[0][:, p * ROWS + sbi * 128:
                               p * ROWS + (sbi + 1) * 128],
                        wo_p[0][p][:, doc * 512:(doc + 1) * 512],
                        start=(p == 0), stop=(p == 3))
                nc.vector.tensor_copy(o0sb[ci][:], pout[:])

            def emit_pass1():
                for sbi in range(4):
                    outt = sb.tile([128, DO], F32, tag="osb", bufs=2,
                                   name=f"outt{sbi}")
                    for doc in range(2):
                        pout = ps.tile([128, 512], F32, tag="ps_a", bufs=2,
                                       name=f"p1_{sbi}_{doc}")
                        for p in range(4):
                            nc.tensor.matmul(
                                pout[:],
                                oTf[1][:, p * ROWS + sbi * 128:
                                       p * ROWS + (sbi + 1) * 128],
                                wo_p[1][p][:, doc * 512:(doc + 1) * 512],
                                start=(p == 0), stop=(p == 3))
                        nc.vector.tensor_add(
                            outt[:, doc * 512:(doc + 1) * 512], pout[:],
                            o0sb[sbi * 2 + doc][:])
                    nc.sync.dma_start(out_d[sbi * 128:(sbi + 1) * 128, :],
                                      outt[:])

            # ---- schedule ----
            # Warm the PE clock while the first DMAs land.
            wrm_f = sb.tile([1, TCH], F32, tag="wrmf", bufs=1)
            nc.vector.memset(wrm_f[:], 1.0)
            wrm = sb.tile([1, TCH], F32R, tag="wrm", bufs=1)
            nc.vector.tensor_copy(wrm[:], wrm_f[:])
            for wi in range(8):
                wps = ps.tile([64, TCH], F32, tag="ps_a", bufs=2,
                              name=f"swarm{wi}")
                nc.tensor.matmul(wps[:], ones_b[0:1, :], wrm[:],
                                 start=True, stop=True)
            # chunks 0-3 fully; batch-0 attention interleaved with the k/v
            # parts of chunks 4-7 (batch-1 q parts are deferred into the
            # otherwise Act-bound batch-1 head-0 region).
            for tci in range(4):
                emit_proj(tci, "qkv")
            units_b0 = [(0, h, qc) for h in range(HPC)
                        for qc in range(S // TCH)]
            for i, tci in enumerate(range(4, 8)):
                for u in units_b0[2 * i:2 * i + 2]:
                    emit_attn(*u)
                emit_proj(tci, "kv")
            emit_proj(4, "q")
            # batch-1 h0 units; A2A(h0); h1 units with pass0 interleaved
            for qc in range(S // TCH):
                emit_attn(1, 0, qc)
                if qc < 3:
                    emit_proj(5 + qc, "q")
            emit_tail()  # flush (1,0,3)'s obc before the h0 collective
            emit_a2a(0)
            emit_oTf(0)
            for qc in range(S // TCH):
                emit_attn(1, 1, qc)
                emit_pass0_chunk(2 * qc)
                emit_pass0_chunk(2 * qc + 1)
            emit_tail()
            emit_a2a(1)
            # keep PE's HAM clock warm across the exposed collective so the
            # odd-heads projection pass starts at 2.4 GHz
            for wi in range(12):
                wps = ps.tile([64, TCH], F32, tag="ps_a", bufs=2,
                              name=f"warm{wi}")
                nc.tensor.matmul(
                    wps[:], last_obc[0][:, 0:64], last_obc[0][:],
                    start=True, stop=True)
            emit_oTf(1)
            emit_pass1()

    nc.compile()
    return nc


def _get_nc():
    if "nc" not in _cache:
        _cache["nc"] = _build()
    return _cache["nc"]


def _in_maps(x, Wq, Wk, Wv, Wo):
    bf16 = ml_dtypes.bfloat16
    xt = np.ascontiguousarray(
        x.reshape(T, D).T.astype(bf16))
    wo = np.ascontiguousarray(Wo.astype(bf16))
    maps = []
    for c in range(NCORES):
        h0, h1 = HPC * c, HPC * c + 1
        maps.append({
            "xt": xt,
            "wq": np.ascontiguousarray(
                np.concatenate([Wq[h0], Wq[h1]], axis=1).astype(bf16)),
            "wk": np.ascontiguousarray(
                np.concatenate([Wk[h0], Wk[h1]], axis=1).astype(bf16)),
            "wv": np.ascontiguousarray(
                np.concatenate([Wv[h0], Wv[h1]], axis=1).astype(bf16)),
            "wo": wo,
        })
    return maps


def kernel(x, Wq, Wk, Wv, Wo, **_):
    nc = _get_nc()
    res = bass_utils.run_bass_kernel_spmd(
        nc, _in_maps(x, Wq, Wk, Wv, Wo), core_ids=list(range(NCORES)))
    out = np.concatenate([res.results[c]["out"] for c in range(NCORES)],
                         axis=0)
    return out.reshape(B, S, DO)
